# revision 1
# baseline (speedup 1.0000x reference)
"""DigitCaps (CapsNet dynamic routing) Trainium2 Bass kernel.

Full computation per batch element b:
    u_hat[r,c,o] = sum_i u[r,i] * W[r,c,i,o]            (einsum)
    b_log = 0; for 3 iters: coef = softmax_c(b_log); s = sum_r coef*u_hat
                v = squash(s); b_log += sum_o u_hat*v
Output: v from last iteration.  Identity used: b_log(t) = u_hat . Vcum(t)
where Vcum = sum of previous v's, so logits are recomputed from Vcum
each iteration instead of accumulated.

Sharding: data-parallel over batch, 512 -> 8 cores x 64.

Per-core layout (P = 128 partitions):
  - einsum operands live in a "spread" layout: r = 16k + m (k in 0..72,
    m in 0..16); partition p(m,i) = (m%4)*32 + (m//4)*8 + i.  The 4
    MMs of 4 consecutive m hit 4 distinct 32-row groups of the PE
    array and run concurrently (tile_position row packing).
  - u_hat is resident in SBUF as bf16 [128 = (rq, b32); 288, 10, 16]
    with partition = rq*32 + b_local (rq = r quartile), per 32-batch
    tile (2 tiles per core).
  - iter-0 s (uniform coef 0.1) is a clean K=128 chunked matmul.
  - routing iters 1..2: z/softmax/s on DVE/ACT; cross-partition-group
    reductions (sum over the 4 rq groups) and the V broadcast to the
    (rq,b) layout are done with tiny constant matmuls (REP/REPT).
"""

import sys

sys.path.insert(0, "/opt/trn_rl_repo")

import functools
from contextlib import ExitStack

import numpy as np

NCORES = 8
B = 64          # batch per core
BT = 32         # batch tile
R = 1152
C = 10
I = 8
O = 16
CO = C * O      # 160
NK = 72         # r-chunks of 16
RQ_K = 18       # k's per r-quartile (288 r's)
RL = 288        # r_loc per quartile
ZCH = 16        # r_locs per routing chunk
NCH = RL // ZCH  # 18
USE_TPOS = True     # pass explicit tile_position on einsum MMs
SKIP_Z = False
SKIP_SM = False
SKIP_S = False
BANK_ALIGN = False  # one PSUM bank per einsum MM output


def _wslice(w):
    return slice(w * 32, (w + 1) * 32)


def build_bass(phase: str = "full"):
    import concourse.bass as bass
    import concourse.tile as tile
    from concourse import bacc, mybir
    from concourse.masks import make_identity

    f32 = mybir.dt.float32
    bf16 = mybir.dt.bfloat16
    AX = mybir.AxisListType
    OP = mybir.AluOpType
    AF = mybir.ActivationFunctionType

    nc = bacc.Bacc(
        "TRN2",
        target_bir_lowering=False,
        debug=False,
        enable_asserts=False,
        num_devices=NCORES,
    )
    u_d = nc.dram_tensor("u", [B, R, I], f32, kind="ExternalInput").ap()
    w_d = nc.dram_tensor("w", [R, C, I, O], f32, kind="ExternalInput").ap()
    v_d = nc.dram_tensor("v", [B, C, O], f32, kind="ExternalOutput").ap()
    m_d = nc.dram_tensor("msk", [128, 4], f32, kind="ExternalInput").ap()

    with tile.TileContext(nc) as tc, ExitStack() as ctx:
        # ---------------- persistent pools ----------------
        consts = ctx.enter_context(tc.tile_pool(name="consts", bufs=1))
        persist = ctx.enter_context(tc.tile_pool(name="persist", bufs=1))

        # uTz[j] holds u^T (partition p = (r%16)*8 + i) with only the
        # rows of m % 4 == j kept, zeros elsewhere -> a K=32 matmul on a
        # 32-aligned window isolates one r.
        uTz0 = persist.tile([128, NK, B], bf16)
        uTz1 = persist.tile([128, NK, B], bf16)
        uTz2 = persist.tile([128, NK, B], bf16)
        uTz3 = persist.tile([128, NK, B], bf16)
        uTz = [uTz0, uTz1, uTz2, uTz3]
        W_sb = persist.tile([128, NK, C, O], bf16)  # 23 KB/part

        def ecopy(which, out_ap, in_ap):
            if which == 0:
                nc.vector.tensor_copy(out_ap, in_ap)
            else:
                nc.scalar.copy(out_ap, in_ap)

        id64 = consts.tile([64, 64], bf16)
        make_identity(nc, id64)
        id32 = consts.tile([32, 32], f32)
        make_identity(nc, id32)

        # REP[b, (q, b')] = 1.0 iff b == b'   (f32, [32, 4, 32])
        REP = consts.tile([32, 4, 32], f32)
        nc.gpsimd.memset(REP[:], 0.0)
        nc.gpsimd.affine_select(
            out=REP[:],
            in_=REP[:],
            compare_op=OP.not_equal,
            fill=1.0,
            base=0,
            pattern=[[0, 4], [-1, 32]],
            channel_multiplier=1,
        )
        REPT = consts.tile([128, 32], f32)

        # ---------------- routing-side pools needed inside prep (iter0) --
        rt = ctx.enter_context(tc.tile_pool(name="rt", bufs=1))
        sm = ctx.enter_context(tc.tile_pool(name="sm", bufs=1))
        logits = rt.tile([128, RL, C], f32)          # 11.5 KB
        coefb = rt.tile([128, RL, C], bf16)          # 5.6 KB
        den = rt.tile([128, RL], f32)
        V_rep = rt.tile([128, C, O], bf16)
        V_exp = rt.tile([128, ZCH, C, O], bf16)   # dense V for 2x TT
        s_acc = rt.tile([128, CO], f32)
        v0 = rt.tile([64, C, O], f32)
        Vcb0 = rt.tile([32, C, O], f32)
        Vcb1 = rt.tile([32, C, O], f32)
        Vcb = [Vcb0, Vcb1]

        def squash(p, s_ap, out_ap, pool):
            # out = |s| / (1 + |s|^2) * s   per (partition, c)
            sq = pool.tile([p, C, O], f32, tag="sqt")
            nc.vector.tensor_mul(sq[:], s_ap, s_ap)
            ssum = pool.tile([p, C], f32, tag="sst")
            nc.vector.tensor_reduce(ssum[:], sq[:], axis=AX.X, op=OP.add)
            norm = pool.tile([p, C], f32, tag="snt")
            nc.scalar.sqrt(norm[:], ssum[:])
            onep = pool.tile([p, C], f32, tag="sot")
            nc.scalar.add(onep[:], ssum[:], 1.0)
            rec = pool.tile([p, C], f32, tag="srt")
            nc.vector.reciprocal(rec[:], onep[:])
            fac = pool.tile([p, C], f32, tag="sft")
            nc.vector.tensor_mul(fac[:], norm[:], rec[:])
            nc.vector.tensor_mul(
                out_ap,
                s_ap,
                fac[:].unsqueeze(2).broadcast_to((p, C, O)),
            )

        def iter0(s0ps):
            s_all = rt.tile([64, C, O], f32)
            nc.scalar.mul(
                s_all[:], s0ps[:].rearrange("p (c o) -> p c o", c=C), 0.1
            )
            squash(64, s_all[:], v0[:], sm)
            nc.vector.tensor_copy(Vcb[0][:], v0[0:32, :, :])
            nc.sync.dma_start(out=Vcb[1][:], in_=v0[32:64, :, :])

        # ---------------- prep phase ----------------
        with ExitStack() as prep:
            pp = prep.enter_context(tc.tile_pool(name="prep", bufs=1))
            wch = prep.enter_context(tc.tile_pool(name="wch", bufs=1))
            ppsum = prep.enter_context(
                tc.tile_pool(name="ppsum", bufs=2, space="PSUM")
            )
            s0_pool = prep.enter_context(
                tc.tile_pool(name="s0psp", bufs=1, space="PSUM")
            )
            s0ps = s0_pool.tile([64, CO], f32)

            # REPT = REP^T via PE
            rps = ppsum.tile([128, 32], f32)
            nc.tensor.transpose(
                rps[:], REP[:].rearrange("b q c -> b (q c)"), id32[:]
            )
            nc.vector.tensor_copy(REPT[:], rps[:])

            # u: load, cast, transpose into spread layout
            u_nat = pp.tile([64, R * I], f32)
            nc.sync.dma_start(
                out=u_nat[:], in_=u_d.rearrange("b r i -> b (r i)")
            )
            u_bf = pp.tile([64, R * I], bf16)
            nc.vector.tensor_copy(u_bf[:], u_nat[:])

            msk = pp.tile([128, 4], f32)
            nc.sync.dma_start(out=msk[:], in_=m_d)
            uT_full = pp.tile([128, NK, B], bf16)
            for kb in range(9):
                pt = ppsum.tile([128, 8, 64], bf16, tag="tp")
                for jj in range(8):
                    k = kb * 8 + jj
                    nc.tensor.transpose(
                        pt[:, jj, :], u_bf[:, k * 128 : (k + 1) * 128], id64[:]
                    )
                ecopy(kb % 2, uT_full[:, kb * 8 : (kb + 1) * 8, :], pt[:])
            for j in range(4):
                nc.vector.tensor_scalar_mul(
                    uTz[j][:].rearrange("p k b -> p (k b)"),
                    uT_full[:].rearrange("p k b -> p (k b)"),
                    msk[:, j : j + 1],
                )

            # W: strided gather into spread layout, cast to bf16
            wst = wch.tile([128, NK, C, O], f32, tag="wst")
            for m in range(16):
                for c in range(C):
                    src = bass.AP(
                        tensor=w_d.tensor,
                        offset=m * (C * I * O) + c * (I * O),
                        ap=[
                            [O, I],                # i (partition dim, 8)
                            [16 * C * I * O, NK],  # k
                            [1, O],                # o
                        ],
                    )
                    deng = nc.sync if (m * C + c) % 2 == 0 else nc.gpsimd
                    deng.dma_start(out=wst[m * I : (m + 1) * I, :, c, :], in_=src)
            ecopy(0, W_sb[0:64, :, :, :], wst[0:64])
            ecopy(1, W_sb[64:128, :, :, :], wst[64:128])

            # iter-0 s matmul chain while the full u^T is still alive:
            # s0 = sum_k uT_full[:,k,:].T @ W_sb[:,k,:]  (all 64 b at once)
            for k in range(NK):
                nc.tensor.matmul(
                    s0ps[:],
                    uT_full[:, k, :],
                    W_sb[:, k, :, :],
                    start=(k == 0),
                    stop=(k == NK - 1),
                )
            if phase != "prep":
                iter0(s0ps)

        # ---------------- main pools ----------------
        big = ctx.enter_context(tc.tile_pool(name="big", bufs=1))
        scratch = ctx.enter_context(tc.tile_pool(name="scratch", bufs=2))
        u_hat = big.tile([128, RL, C, O], bf16)      # 92 KB/part

        # ---------------- einsum: u_hat per batch tile ----------------
        def einsum_tile(bt, mm_psum):
            # storage index within a 16-block: rs = j*4 + w (the 4
            # w-concurrent MMs land on consecutive r slots).  Any r
            # permutation is fine: routing is symmetric in r.
            # Matmul PSUM outputs must start at a bank boundary: each
            # row-group w gets its own bank; the 4 col-groups (rq) share
            # the bank at offset 0 on disjoint partition quarters.
            for kl in range(RQ_K):
                for j in range(4):
                    pe_ps = mm_psum.tile([128, 4, 512], f32, tag="pe")
                    for rq in range(4):
                        k = rq * RQ_K + kl
                        for w in range(4):
                            nc.tensor.matmul(
                                pe_ps[rq * 32 : (rq + 1) * 32, w, 0:CO],
                                uTz[j][_wslice(w), k, bt * BT : (bt + 1) * BT],
                                W_sb[_wslice(w), k, :, :],
                                start=True,
                                stop=True,
                                tile_position=(w * 32, rq * 32),
                            )
                    rs0 = 16 * kl + j * 4
                    dst = u_hat[:, rs0 : rs0 + 4, :, :].rearrange(
                        "p r c o -> p r (c o)"
                    )
                    ecopy(j % 2, dst, pe_ps[:, :, 0:CO])

        # ---------------- routing iteration ----------------
        def routing_iter(bt, t, it_psum):
            # V broadcast to (rq, b) layout:  V_rep = REP^T-ish matmul
            vps = it_psum.tile([128, CO], f32, tag="vrep")
            nc.tensor.matmul(
                vps[:],
                REP[:].rearrange("b q c -> b (q c)"),
                Vcb[bt][:].rearrange("p c o -> p (c o)"),
                start=True,
                stop=True,
            )
            nc.vector.tensor_copy(
                V_rep[:].rearrange("p c o -> p (c o)"), vps[:]
            )
            # dense V replica over the r-chunk: makes the z-product a
            # dense bf16 tensor_tensor (2x mode) instead of broadcast (1x)
            nc.scalar.copy(
                V_exp[:],
                V_rep[:].unsqueeze(1).broadcast_to((128, ZCH, C, O)),
            )
            # z-pass: logits = sum_o u_hat * V.  Products alternate
            # DVE/GPSIMD so reduces (DVE-only) overlap products.
            for rc in range(NCH if not SKIP_Z else 0):
                pr = scratch.tile([128, ZCH, C, O], bf16, tag="zpr")
                peng = nc.gpsimd if rc % 2 == 0 else nc.vector
                peng.tensor_mul(
                    pr[:],
                    u_hat[:, rc * ZCH : (rc + 1) * ZCH, :, :],
                    V_exp[:],
                )
                nc.vector.tensor_reduce(
                    logits[:, rc * ZCH : (rc + 1) * ZCH, :],
                    pr[:],
                    axis=AX.X,
                    op=OP.add,
                )
            # softmax over c.  No max-subtraction: |logits| <~ 60 is far
            # inside fp32 exp range, softmax is shift-invariant.
            if SKIP_SM:
                nc.vector.tensor_copy(coefb[:], logits[:])
            else:
                nc.scalar.activation(
                    logits[:].rearrange("p r c -> p (r c)"),
                    logits[:].rearrange("p r c -> p (r c)"),
                    AF.Exp,
                )
                nc.vector.tensor_reduce(den[:], logits[:], axis=AX.X, op=OP.add)
                nc.vector.reciprocal(den[:], den[:])
                nc.vector.tensor_mul(
                    coefb[:],
                    logits[:],
                    den[:].unsqueeze(2).broadcast_to((128, RL, C)),
                )
            # s-pass: s_acc = sum_rloc coef * u_hat
            nc.gpsimd.memset(s_acc[:], 0.0)
            for rc in range(NCH if not SKIP_S else 0):
                pr2 = scratch.tile([128, ZCH, C, O], bf16, tag="spr")
                peng = nc.gpsimd if rc % 2 == 1 else nc.vector
                peng.tensor_mul(
                    pr2[:],
                    u_hat[:, rc * ZCH : (rc + 1) * ZCH, :, :],
                    coefb[:, rc * ZCH : (rc + 1) * ZCH, :]
                    .unsqueeze(3)
                    .broadcast_to((128, ZCH, C, O)),
                )
                red = sm.tile([128, CO], f32, tag="red")
                nc.vector.tensor_reduce(
                    red[:].rearrange("p (c o) -> p c o", c=C),
                    pr2[:].rearrange("p r c o -> p c o r"),
                    axis=AX.X,
                    op=OP.add,
                )
                nc.vector.tensor_add(s_acc[:], s_acc[:], red[:])
            # combine the 4 rq groups: s_bt[b] = sum_rq s_acc[rq*32+b]
            scps = it_psum.tile([32, CO], f32, tag="comb")
            nc.tensor.matmul(
                scps[:], REPT[:], s_acc[:], start=True, stop=True
            )
            s_bt = sm.tile([32, C, O], f32, tag="sbt")
            nc.vector.tensor_copy(
                s_bt[:].rearrange("p c o -> p (c o)"), scps[:]
            )
            v_t = sm.tile([32, C, O], f32, tag="vt")
            squash(32, s_bt[:], v_t[:], sm)
            if t == 1:
                nc.vector.tensor_add(Vcb[bt][:], Vcb[bt][:], v_t[:])
            else:
                nc.sync.dma_start(
                    out=v_d[bt * BT : (bt + 1) * BT, :, :], in_=v_t[:]
                )

        mm_bufs = 1 if BANK_ALIGN else 2
        if phase == "prep":
            pass
        elif phase == "einsum":
            with ExitStack() as es:
                mm_psum = es.enter_context(
                    tc.tile_pool(name="mmps0", bufs=mm_bufs, space="PSUM")
                )
                einsum_tile(0, mm_psum)
        elif phase == "iter0":
            with ExitStack() as es:
                mm_psum = es.enter_context(
                    tc.tile_pool(name="mmps0", bufs=mm_bufs, space="PSUM")
                )
                einsum_tile(0, mm_psum)
                nc.sync.dma_start(out=v_d[0:BT, :, :], in_=v0[0:32, :, :])
        else:
            for bt in range(2):
                with ExitStack() as es:
                    mm_psum = es.enter_context(
                        tc.tile_pool(name=f"mmps{bt}", bufs=mm_bufs, space="PSUM")
                    )
                    einsum_tile(bt, mm_psum)
                with ExitStack() as es:
                    it_psum = es.enter_context(
                        tc.tile_pool(name=f"itps{bt}", bufs=2, space="PSUM")
                    )
                    for t in (1, 2):
                        routing_iter(bt, t, it_psum)

    nc.compile()
    return nc


@functools.cache
def _get_nc():
    return build_bass()


def make_mask() -> np.ndarray:
    p = np.arange(128)
    j = (p // I) % 4
    return (j[:, None] == np.arange(4)[None, :]).astype(np.float32)


def kernel(u: np.ndarray, W: np.ndarray) -> np.ndarray:
    from concourse import bass_utils

    nc = _get_nc()
    W4 = np.ascontiguousarray(W.reshape(R, C, I, O)).astype(np.float32)
    msk = make_mask()
    in_maps = [
        {
            "u": np.ascontiguousarray(u[i * B : (i + 1) * B]).astype(np.float32),
            "w": W4,
            "msk": msk,
        }
        for i in range(NCORES)
    ]
    res = bass_utils.run_bass_kernel_spmd(
        nc, in_maps, core_ids=list(range(NCORES))
    )
    return np.concatenate([r["v"] for r in res.results], axis=0)



# revision 4
# speedup vs baseline: 1.2672x; 1.2672x over previous
"""DigitCaps (CapsNet dynamic routing) Trainium2 Bass kernel.

Full computation per batch element b:
    u_hat[r,c,o] = sum_i u[r,i] * W[r,c,i,o]            (einsum)
    b_log = 0; for 3 iters: coef = softmax_c(b_log); s = sum_r coef*u_hat
                v = squash(s); b_log += sum_o u_hat*v
Output: v from last iteration.  Identity used: b_log(t) = u_hat . Vcum(t)
where Vcum = sum of previous v's, so logits are recomputed from Vcum
each iteration instead of accumulated.

Sharding: data-parallel over batch, 512 -> 8 cores x 64.

Key cost-model-driven choices vs the naive version:
  - All operand layouts (u^T spread, masked uTz variants, W spread) are
    packed on the HOST and DMAed as single contiguous bf16 blocks: no
    strided gather DMAs, no on-device transposes or masking.
  - The z/s reductions run as halving ADD-trees in fp16 (TensorTensor,
    DVE 2x mode) instead of TensorReduce (which has no fast modes).
  - s-product keeps 2x mode via coef2 (coefficients duplicated in o-pairs
    so the broadcast AP stays packed in the last dim).
  - V broadcast and the rq-group combine use partition-offset DVE adds,
    not PE/PSUM, so PSUM belongs entirely to the einsum and the next
    batch-tile's einsum overlaps the tail of the current routing.
  - Elementwise work is split DVE (2x) / GPSIMD by span to balance
    engine occupancy; einsum PSUM evictions rotate DVE/ACT/GPSIMD.
"""

import sys

sys.path.insert(0, "/opt/trn_rl_repo")

import functools
from contextlib import ExitStack

import numpy as np

NCORES = 8
B = 64          # batch per core
BT = 32         # batch tile
R = 1152
C = 10
I = 8
O = 16
CO = C * O      # 160
NK = 72         # r-chunks of 16
RQ_K = 18       # k's per r-quartile
RL = 288        # r_loc per quartile (per partition)
RSP = 32        # rl span for routing passes
NSP = RL // RSP  # 9
POOL_SPANS = (7, 8)   # spans assigned to gpsimd (products + trees)
NORM_SPLIT = 192      # rl rows on DVE for the coef2 normalize


def _wslice(w):
    return slice(w * 32, (w + 1) * 32)


def build_bass(phase: str = "full"):
    import concourse.bass as bass
    import concourse.tile as tile
    from concourse import bacc, mybir

    f32 = mybir.dt.float32
    bf16 = mybir.dt.bfloat16
    f16 = mybir.dt.float16
    AX = mybir.AxisListType
    OP = mybir.AluOpType
    AF = mybir.ActivationFunctionType

    nc = bacc.Bacc(
        "TRN2",
        target_bir_lowering=False,
        debug=False,
        enable_asserts=False,
        num_devices=NCORES,
    )
    # Host-packed operands (bf16, contiguous):
    #   ut   [128, NK, B]    u^T spread: partition p = 8*m + i, r = 16k + m
    #   utz  [4, 128, NK, B] ut with only rows m%4 == j kept
    #   wsb  [128, NK, C, O] W spread: same partition map
    ut_d = nc.dram_tensor("ut", [128, NK, B], bf16, kind="ExternalInput").ap()
    utz_d = nc.dram_tensor("utz", [4, 128, NK, B], bf16, kind="ExternalInput").ap()
    w_d = nc.dram_tensor("wsb", [128, NK, C, O], bf16, kind="ExternalInput").ap()
    v_d = nc.dram_tensor("v", [B, C, O], f32, kind="ExternalOutput").ap()

    with tile.TileContext(nc) as tc, ExitStack() as ctx:
        # ---------------- persistent pools ----------------
        persist = ctx.enter_context(tc.tile_pool(name="persist", bufs=1))
        uTz0 = persist.tile([128, NK, B], bf16)
        uTz1 = persist.tile([128, NK, B], bf16)
        uTz2 = persist.tile([128, NK, B], bf16)
        uTz3 = persist.tile([128, NK, B], bf16)
        uTz = [uTz0, uTz1, uTz2, uTz3]
        W_sb = persist.tile([128, NK, C, O], bf16)

        rt = ctx.enter_context(tc.tile_pool(name="rt", bufs=1))
        logitsH = rt.tile([128, RL, C], f16)
        E = rt.tile([128, RL, C], bf16)
        den = rt.tile([128, RL], f32)
        coef2 = rt.tile([128, RL, C, 2], bf16)
        V_rep = rt.tile([128, C, O], bf16)
        s_pair = rt.tile([128, 2, CO], f16)
        cmb = rt.tile([32, 2, CO], f16)
        v0 = rt.tile([64, C, O], f32)
        Vcb0 = rt.tile([32, C, O], f32)
        Vcb1 = rt.tile([32, C, O], f32)
        Vcb = [Vcb0, Vcb1]
        sm = ctx.enter_context(tc.tile_pool(name="sm", bufs=1))

        def ecopy(which, out_ap, in_ap):
            if which == 0:
                nc.vector.tensor_copy(out_ap, in_ap)
            elif which == 1:
                nc.scalar.copy(out_ap, in_ap)
            else:
                nc.gpsimd.tensor_copy(out_ap, in_ap)

        def squash(p, s_ap, out_ap, pool):
            # out = |s| / (1 + |s|^2) * s   per (partition, c)
            sq = pool.tile([p, C, O], f32, tag="sqt")
            nc.vector.tensor_mul(sq[:], s_ap, s_ap)
            ssum = pool.tile([p, C], f32, tag="sst")
            nc.vector.tensor_reduce(ssum[:], sq[:], axis=AX.X, op=OP.add)
            norm = pool.tile([p, C], f32, tag="snt")
            nc.scalar.sqrt(norm[:], ssum[:])
            onep = pool.tile([p, C], f32, tag="sot")
            nc.scalar.add(onep[:], ssum[:], 1.0)
            rec = pool.tile([p, C], f32, tag="srt")
            nc.vector.reciprocal(rec[:], onep[:])
            fac = pool.tile([p, C], f32, tag="sft")
            nc.vector.tensor_mul(fac[:], norm[:], rec[:])
            nc.vector.tensor_mul(
                out_ap,
                s_ap,
                fac[:].unsqueeze(2).broadcast_to((p, C, O)),
            )

        def iter0(s0ps):
            s_all = rt.tile([64, C, O], f32)
            nc.scalar.mul(
                s_all[:], s0ps[:].rearrange("p (c o) -> p c o", c=C), 0.1
            )
            squash(64, s_all[:], v0[:], sm)
            nc.vector.tensor_copy(Vcb[0][:], v0[0:32, :, :])
            nc.scalar.copy(Vcb[1][:], v0[32:64, :, :])

        # ---------------- prep phase ----------------
        with ExitStack() as prep:
            pp = prep.enter_context(tc.tile_pool(name="prep", bufs=1))
            s0_pool = prep.enter_context(
                tc.tile_pool(name="s0psp", bufs=1, space="PSUM")
            )
            s0ps = s0_pool.tile([64, CO], f32)

            uT_full = pp.tile([128, NK, B], bf16)
            nc.sync.dma_start(out=uT_full[:], in_=ut_d)
            nc.scalar.dma_start(out=W_sb[:], in_=w_d)
            for j in range(4):
                deng = nc.sync if j % 2 == 0 else nc.scalar
                deng.dma_start(out=uTz[j][:], in_=utz_d[j])

            # iter-0 s matmul chain: s0 = sum_k uT_full[:,k,:].T @ W_sb[:,k]
            for k in range(NK):
                nc.tensor.matmul(
                    s0ps[:],
                    uT_full[:, k, :],
                    W_sb[:, k, :, :],
                    start=(k == 0),
                    stop=(k == NK - 1),
                )
            if phase != "prep":
                iter0(s0ps)

        # ---------------- main pools ----------------
        big = ctx.enter_context(tc.tile_pool(name="big", bufs=1))
        scratch = ctx.enter_context(tc.tile_pool(name="scratch", bufs=2))
        u_hat = big.tile([128, RL, C, O], bf16)      # 90 KB/part

        # ---------------- einsum: u_hat per batch tile ----------------
        def einsum_tile(bt, mm_psum):
            # Each MM isolates one r via the zero-masked uTz rows; the 4
            # row-groups (w) and 4 col-groups (rq) tile the PE array.
            for kl in range(RQ_K):
                for j in range(4):
                    pe_ps = mm_psum.tile([128, 4, 512], f32, tag="pe")
                    for rq in range(4):
                        k = rq * RQ_K + kl
                        for w in range(4):
                            nc.tensor.matmul(
                                pe_ps[rq * 32 : (rq + 1) * 32, w, 0:CO],
                                uTz[j][_wslice(w), k, bt * BT : (bt + 1) * BT],
                                W_sb[_wslice(w), k, :, :],
                                start=True,
                                stop=True,
                                tile_position=(w * 32, rq * 32),
                            )
                    rs0 = 16 * kl + j * 4
                    dst = u_hat[:, rs0 : rs0 + 4, :, :].rearrange(
                        "p r c o -> p r (c o)"
                    )
                    ecopy((kl * 4 + j) % 3, dst, pe_ps[:, :, 0:CO])

        # ---------------- routing iteration ----------------
        def vrep_update(bt):
            # V_rep[rq*32+b] = Vcb[bt][b] for all 4 rq groups (cast to bf16)
            nc.vector.tensor_copy(V_rep[0:32, :, :], Vcb[bt][:])
            nc.scalar.copy(V_rep[32:64, :, :], Vcb[bt][:])
            nc.vector.tensor_copy(V_rep[64:96, :, :], Vcb[bt][:])
            nc.scalar.copy(V_rep[96:128, :, :], Vcb[bt][:])

        def routing_iter(bt, t):
            vrep_update(bt)
            # ---- z-pass: logits[p, rl, c] = sum_o u_hat * V_rep ----
            for sp in range(NSP):
                eng = nc.gpsimd if sp in POOL_SPANS else nc.vector
                rl0 = sp * RSP
                pr = scratch.tile([128, RSP, C, O], f16, tag="pr")
                eng.tensor_mul(
                    pr[:],
                    u_hat[:, rl0 : rl0 + RSP, :, :],
                    V_rep[:].unsqueeze(1).broadcast_to((128, RSP, C, O)),
                )
                w = O
                while w > 2:
                    h = w // 2
                    eng.tensor_add(
                        pr[:, :, :, 0:h], pr[:, :, :, 0:h], pr[:, :, :, h:w]
                    )
                    w = h
                eng.tensor_add(
                    logitsH[:, rl0 : rl0 + RSP, :],
                    pr[:, :, :, 0],
                    pr[:, :, :, 1],
                )
            # ---- softmax over c (no max-shift: |z| << fp range) ----
            nc.scalar.activation(
                E[:].rearrange("p r c -> p (r c)"),
                logitsH[:].rearrange("p r c -> p (r c)"),
                AF.Exp,
            )
            nc.vector.tensor_reduce(den[:], E[:], axis=AX.X, op=OP.add)
            nc.vector.reciprocal(den[:], den[:])
            # coef2[p, rl, c, 0:2] = E * den^-1 duplicated in o-pairs
            ns = NORM_SPLIT
            nc.vector.tensor_mul(
                coef2[:, 0:ns, :, :],
                E[:, 0:ns, :].unsqueeze(3).broadcast_to((128, ns, C, 2)),
                den[:, 0:ns].unsqueeze(2).unsqueeze(3).broadcast_to(
                    (128, ns, C, 2)
                ),
            )
            nc.gpsimd.tensor_mul(
                coef2[:, ns:RL, :, :],
                E[:, ns:RL, :].unsqueeze(3).broadcast_to((128, RL - ns, C, 2)),
                den[:, ns:RL].unsqueeze(2).unsqueeze(3).broadcast_to(
                    (128, RL - ns, C, 2)
                ),
            )
            # ---- s-pass: s_pair[p, 0:2, co] = sum_rl coef * u_hat ----
            for sp in range(NSP):
                eng = nc.gpsimd if sp in POOL_SPANS else nc.vector
                rl0 = sp * RSP
                pr2 = scratch.tile([128, RSP, C, O], f16, tag="pr")
                eng.tensor_mul(
                    pr2[:].rearrange("p r c (e two) -> p (r c) e two", two=2),
                    u_hat[:, rl0 : rl0 + RSP, :, :].rearrange(
                        "p r c (e two) -> p (r c) e two", two=2
                    ),
                    coef2[:, rl0 : rl0 + RSP, :, :]
                    .rearrange("p r c two -> p (r c) two")
                    .unsqueeze(2)
                    .broadcast_to((128, RSP * C, O // 2, 2)),
                )
                w = RSP
                prv = pr2[:].rearrange("p r c o -> p r (c o)")
                while w > 2:
                    h = w // 2
                    eng.tensor_add(prv[:, 0:h, :], prv[:, 0:h, :], prv[:, h:w, :])
                    w = h
                if sp == 0:
                    nc.vector.tensor_copy(s_pair[:], prv[:, 0:2, :])
                else:
                    eng2 = nc.vector if sp not in POOL_SPANS else nc.gpsimd
                    eng2.tensor_add(s_pair[:], s_pair[:], prv[:, 0:2, :])
            # ---- combine 4 rq partition groups + o-pairs -> s_bt [32,C,O]
            nc.vector.tensor_add(cmb[:], s_pair[0:32, :, :], s_pair[32:64, :, :])
            nc.vector.tensor_add(cmb[:], cmb[:], s_pair[64:96, :, :])
            nc.vector.tensor_add(cmb[:], cmb[:], s_pair[96:128, :, :])
            s_bt = sm.tile([32, C, O], f32, tag="sbt")
            nc.vector.tensor_add(
                s_bt[:].rearrange("p c o -> p (c o)"), cmb[:, 0, :], cmb[:, 1, :]
            )
            v_t = sm.tile([32, C, O], f32, tag="vt")
            squash(32, s_bt[:], v_t[:], sm)
            if t == 1:
                nc.vector.tensor_add(Vcb[bt][:], Vcb[bt][:], v_t[:])
            else:
                nc.sync.dma_start(
                    out=v_d[bt * BT : (bt + 1) * BT, :, :], in_=v_t[:]
                )

        if phase == "prep":
            pass
        elif phase == "einsum":
            with ExitStack() as es:
                mm_psum = es.enter_context(
                    tc.tile_pool(name="mmps0", bufs=2, space="PSUM")
                )
                einsum_tile(0, mm_psum)
        elif phase == "iter0":
            with ExitStack() as es:
                mm_psum = es.enter_context(
                    tc.tile_pool(name="mmps0", bufs=2, space="PSUM")
                )
                einsum_tile(0, mm_psum)
                nc.sync.dma_start(out=v_d[0:BT, :, :], in_=v0[0:32, :, :])
        else:
            for bt in range(2):
                with ExitStack() as es:
                    mm_psum = es.enter_context(
                        tc.tile_pool(name=f"mmps{bt}", bufs=2, space="PSUM")
                    )
                    einsum_tile(bt, mm_psum)
                for t in (1, 2):
                    routing_iter(bt, t)

    nc.compile()
    return nc


@functools.cache
def _get_nc():
    return build_bass()


@functools.cache
def _pack_cache():
    return {}


def _pack_inputs(u: np.ndarray, W: np.ndarray):
    import ml_dtypes

    bf = ml_dtypes.bfloat16
    # u: [B, R, I] per core slice -> ut [128=(8m+i), NK, B]
    # W: [R, C, I, O] -> wsb [128=(8m+i), NK, C, O]
    W5 = W.reshape(NK, 16, C, I, O)
    wsb = np.ascontiguousarray(
        W5.transpose(1, 3, 0, 2, 4).reshape(128, NK, C, O)
    ).astype(bf)
    m = (np.arange(128) // I)
    masks = [(m % 4 == j).astype(np.float32)[:, None, None] for j in range(4)]

    def pack_u(u_core):
        u5 = u_core.reshape(B, NK, 16, I)
        ut = np.ascontiguousarray(
            u5.transpose(2, 3, 1, 0).reshape(128, NK, B)
        )
        utz = np.stack([ut * msk for msk in masks], axis=0)
        return ut.astype(bf), utz.astype(bf)

    return wsb, pack_u


def kernel(u: np.ndarray, W: np.ndarray) -> np.ndarray:
    from concourse import bass_utils

    nc = _get_nc()
    W4 = np.ascontiguousarray(W.reshape(R, C, I, O)).astype(np.float32)
    wsb, pack_u = _pack_inputs(u, W4)
    in_maps = []
    for i in range(NCORES):
        ut, utz = pack_u(
            np.ascontiguousarray(u[i * B : (i + 1) * B]).astype(np.float32)
        )
        in_maps.append({"ut": ut, "utz": utz, "wsb": wsb})
    res = bass_utils.run_bass_kernel_spmd(
        nc, in_maps, core_ids=list(range(NCORES))
    )
    return np.concatenate([r["v"] for r in res.results], axis=0)


# revision 11
# speedup vs baseline: 1.6551x; 1.3061x over previous
"""DigitCaps (CapsNet dynamic routing) Trainium2 Bass kernel.

Full computation per batch element b:
    u_hat[r,c,o] = sum_i u[r,i] * W[r,c,i,o]            (einsum)
    b_log = 0; for 3 iters: coef = softmax_c(b_log); s = sum_r coef*u_hat
                v = squash(s); b_log += sum_o u_hat*v
Output: v from last iteration.  Identity used: b_log(t) = u_hat . Vcum(t)
where Vcum = sum of previous v's, so logits are recomputed from Vcum
each iteration instead of accumulated.

Sharding: data-parallel over batch, 512 -> 8 cores x 64.

Key cost-model-driven choices vs the naive version:
  - All operand layouts (u^T spread, masked uTz variants, W spread) are
    packed on the HOST and DMAed as single contiguous bf16 blocks: no
    strided gather DMAs, no on-device transposes or masking.
  - The z/s reductions run as halving ADD-trees in fp16 (TensorTensor,
    DVE 2x mode) instead of TensorReduce (which has no fast modes).
  - s-product keeps 2x mode via coef2 (coefficients duplicated in o-pairs
    so the broadcast AP stays packed in the last dim).
  - V broadcast and the rq-group combine use partition-offset DVE adds,
    not PE/PSUM, so PSUM belongs entirely to the einsum and the next
    batch-tile's einsum overlaps the tail of the current routing.
  - Elementwise work is split DVE (2x) / GPSIMD by span to balance
    engine occupancy; einsum PSUM evictions rotate DVE/ACT/GPSIMD.
"""

import sys

sys.path.insert(0, "/opt/trn_rl_repo")

import functools
from contextlib import ExitStack

import numpy as np

NCORES = 8
B = 64          # batch per core
BT = 32         # batch tile
R = 1152
C = 10
I = 8
O = 16
CO = C * O      # 160
NK = 72         # r-chunks of 16
RQ_K = 18       # k's per r-quartile
RL = 288        # r_loc per quartile (per partition)
RSP = 32        # rl span for routing passes
NSP = RL // RSP  # 9
POOL_SPANS = (7, 8)   # spans assigned to gpsimd (products + trees)
NORM_SPLIT = 192      # rl rows on DVE for the coef2 normalize


def _wslice(w):
    return slice(w * 32, (w + 1) * 32)


def build_bass(phase: str = "full"):
    import concourse.bass as bass
    import concourse.tile as tile
    from concourse import bacc, mybir

    f32 = mybir.dt.float32
    bf16 = mybir.dt.bfloat16
    f16 = mybir.dt.float16
    AX = mybir.AxisListType
    OP = mybir.AluOpType
    AF = mybir.ActivationFunctionType

    nc = bacc.Bacc(
        "TRN2",
        target_bir_lowering=False,
        debug=False,
        enable_asserts=False,
        num_devices=NCORES,
    )
    # Host-packed operands (bf16, contiguous):
    #   ut   [128, NK, B]    u^T spread: partition p = 8*m + i, r = 16k + m
    #   utz  [4, 128, NK, B] ut with only rows m%4 == j kept
    #   wsb  [128, NK, C, O] W spread: same partition map
    ut_d = nc.dram_tensor("ut", [128, NK, B], bf16, kind="ExternalInput").ap()
    utz_d = nc.dram_tensor("utz", [4, 128, NK, B], bf16, kind="ExternalInput").ap()
    w_d = nc.dram_tensor("wsb", [128, NK, C, O], bf16, kind="ExternalInput").ap()
    v_d = nc.dram_tensor("v", [B, C, O], f32, kind="ExternalOutput").ap()

    with tile.TileContext(nc) as tc, ExitStack() as ctx:
        # ---------------- persistent pools ----------------
        persist = ctx.enter_context(tc.tile_pool(name="persist", bufs=1))
        uTz0 = persist.tile([128, NK, B], bf16)
        uTz1 = persist.tile([128, NK, B], bf16)
        uTz2 = persist.tile([128, NK, B], bf16)
        uTz3 = persist.tile([128, NK, B], bf16)
        uTz = [uTz0, uTz1, uTz2, uTz3]
        W_sb = persist.tile([128, NK, C, O], bf16)

        rt = ctx.enter_context(tc.tile_pool(name="rt", bufs=1))
        logitsH = rt.tile([128, RL, C], f16)
        E = rt.tile([128, RL, C], bf16)
        den = rt.tile([128, RL], f32)
        coef2 = rt.tile([128, RL, C, 2], bf16)
        V_rep = rt.tile([128, C, O], bf16)
        s_pairD = rt.tile([128, 2, CO], f16)
        s_pairP = rt.tile([128, 2, CO], f16)
        cmb = rt.tile([32, 2, CO], f16)
        v0 = rt.tile([64, C, O], f32)
        Vcb0 = rt.tile([32, C, O], f32)
        Vcb1 = rt.tile([32, C, O], f32)
        Vcb = [Vcb0, Vcb1]
        sm = ctx.enter_context(tc.tile_pool(name="sm", bufs=1))

        def ecopy(which, out_ap, in_ap):
            if which == 0:
                nc.vector.tensor_copy(out_ap, in_ap)
            elif which == 1:
                nc.scalar.copy(out_ap, in_ap)
            else:
                nc.gpsimd.tensor_copy(out_ap, in_ap)

        def squash(p, s_ap, out_ap, pool):
            # out = |s| / (1 + |s|^2) * s   per (partition, c)
            sq = pool.tile([p, C, O], f32, tag="sqt")
            nc.vector.tensor_mul(sq[:], s_ap, s_ap)
            ssum = pool.tile([p, C], f32, tag="sst")
            nc.vector.tensor_reduce(ssum[:], sq[:], axis=AX.X, op=OP.add)
            # sqrt(x) = exp(0.5*ln(x)): Ln and Exp share one activation
            # table with the softmax Exp, avoiding ACT table reloads.
            norm = pool.tile([p, C], f32, tag="snt")
            nc.scalar.activation(norm[:], ssum[:], AF.Ln)
            nc.scalar.activation(norm[:], norm[:], AF.Exp, scale=0.5)
            onep = pool.tile([p, C], f32, tag="sot")
            nc.scalar.add(onep[:], ssum[:], 1.0)
            rec = pool.tile([p, C], f32, tag="srt")
            nc.vector.reciprocal(rec[:], onep[:])
            fac = pool.tile([p, C], f32, tag="sft")
            nc.vector.tensor_mul(fac[:], norm[:], rec[:])
            nc.vector.tensor_mul(
                out_ap,
                s_ap,
                fac[:].unsqueeze(2).broadcast_to((p, C, O)),
            )

        def iter0(s0ps):
            s_all = rt.tile([64, C, O], f32)
            nc.scalar.mul(
                s_all[:], s0ps[:].rearrange("p (c o) -> p c o", c=C), 0.1
            )
            squash(64, s_all[:], v0[:], sm)
            nc.vector.tensor_copy(Vcb[0][:], v0[0:32, :, :])
            nc.scalar.copy(Vcb[1][:], v0[32:64, :, :])

        # ---------------- prep phase ----------------
        with ExitStack() as prep:
            pp = prep.enter_context(tc.tile_pool(name="prep", bufs=1))
            s0_pool = prep.enter_context(
                tc.tile_pool(name="s0psp", bufs=1, space="PSUM")
            )
            s0ps = s0_pool.tile([64, CO], f32)

            uT_full = pp.tile([128, NK, B], bf16)
            nc.sync.dma_start(out=uT_full[:], in_=ut_d)
            nc.scalar.dma_start(out=W_sb[:], in_=w_d)
            for j in range(4):
                deng = nc.sync if j % 2 == 0 else nc.scalar
                deng.dma_start(out=uTz[j][:], in_=utz_d[j])

            # iter-0 s matmul chain: s0 = sum_k uT_full[:,k,:].T @ W_sb[:,k]
            for k in range(NK):
                nc.tensor.matmul(
                    s0ps[:],
                    uT_full[:, k, :],
                    W_sb[:, k, :, :],
                    start=(k == 0),
                    stop=(k == NK - 1),
                )
            if phase != "prep":
                iter0(s0ps)

        # ---------------- main pools ----------------
        big = ctx.enter_context(tc.tile_pool(name="big", bufs=1))
        scratch = ctx.enter_context(tc.tile_pool(name="scratch", bufs=1))
        u_hat = big.tile([128, RL, C, O], bf16)      # 90 KB/part

        # ---------------- einsum: u_hat per batch tile ----------------
        def einsum_tile(bt, mm_psum):
            # Each MM isolates one r via the zero-masked uTz rows; the 4
            # row-groups (w) and 4 col-groups (rq) tile the PE array.
            for kl in range(RQ_K):
                for j in range(4):
                    pe_ps = mm_psum.tile([128, 4, 512], f32, tag="pe")
                    for rq in range(4):
                        k = rq * RQ_K + kl
                        for w in range(4):
                            nc.tensor.matmul(
                                pe_ps[rq * 32 : (rq + 1) * 32, w, 0:CO],
                                uTz[j][_wslice(w), k, bt * BT : (bt + 1) * BT],
                                W_sb[_wslice(w), k, :, :],
                                start=True,
                                stop=True,
                                tile_position=(w * 32, rq * 32),
                            )
                    rs0 = 16 * kl + j * 4
                    dst = u_hat[:, rs0 : rs0 + 4, :, :].rearrange(
                        "p r c o -> p r (c o)"
                    )
                    ecopy((kl * 4 + j) % 3, dst, pe_ps[:, :, 0:CO])

        # ---------------- routing iteration ----------------
        def vrep_update(bt):
            # V_rep[rq*32+b] = Vcb[bt][b] for all 4 rq groups (cast to bf16)
            nc.vector.tensor_copy(V_rep[0:32, :, :], Vcb[bt][:])
            nc.scalar.copy(V_rep[32:64, :, :], Vcb[bt][:])
            nc.vector.tensor_copy(V_rep[64:96, :, :], Vcb[bt][:])
            nc.scalar.copy(V_rep[96:128, :, :], Vcb[bt][:])

        # Pool spans are emitted first: gpsimd is ~4x slower per element,
        # so its spans define the pass tail and must start at pass begin.
        SPAN_ORDER = list(POOL_SPANS) + [
            sp for sp in range(NSP) if sp not in POOL_SPANS
        ]

        def routing_iter(bt, t):
            vrep_update(bt)
            # ---- z-pass: logits[p, rl, c] = sum_o u_hat * V_rep ----
            for sp in SPAN_ORDER:
                pool_sp = sp in POOL_SPANS
                eng = nc.gpsimd if pool_sp else nc.vector
                rl0 = sp * RSP
                pr = scratch.tile(
                    [128, RSP, C, O], f16, tag="prp" if pool_sp else "prd"
                )
                eng.tensor_mul(
                    pr[:],
                    u_hat[:, rl0 : rl0 + RSP, :, :],
                    V_rep[:].unsqueeze(1).broadcast_to((128, RSP, C, O)),
                )
                w = O
                while w > 2:
                    h = w // 2
                    eng.tensor_add(
                        pr[:, :, :, 0:h], pr[:, :, :, 0:h], pr[:, :, :, h:w]
                    )
                    w = h
                eng.tensor_add(
                    logitsH[:, rl0 : rl0 + RSP, :],
                    pr[:, :, :, 0],
                    pr[:, :, :, 1],
                )
            # ---- softmax over c (no max-shift: |z| << fp range) ----
            nc.scalar.activation(
                E[:].rearrange("p r c -> p (r c)"),
                logitsH[:].rearrange("p r c -> p (r c)"),
                AF.Exp,
            )
            nc.vector.tensor_reduce(den[:], E[:], axis=AX.X, op=OP.add)
            nc.vector.reciprocal(den[:], den[:])
            # coef2[p, rl, c, 0:2] = E * den^-1 duplicated in o-pairs
            ns = NORM_SPLIT
            nc.vector.tensor_mul(
                coef2[:, 0:ns, :, :],
                E[:, 0:ns, :].unsqueeze(3).broadcast_to((128, ns, C, 2)),
                den[:, 0:ns].unsqueeze(2).unsqueeze(3).broadcast_to(
                    (128, ns, C, 2)
                ),
            )
            nc.gpsimd.tensor_mul(
                coef2[:, ns:RL, :, :],
                E[:, ns:RL, :].unsqueeze(3).broadcast_to((128, RL - ns, C, 2)),
                den[:, ns:RL].unsqueeze(2).unsqueeze(3).broadcast_to(
                    (128, RL - ns, C, 2)
                ),
            )
            # ---- s-pass: s_pair*[p, 0:2, co] = sum_rl coef * u_hat ----
            firstD = True
            firstP = True
            for sp in SPAN_ORDER:
                pool_sp = sp in POOL_SPANS
                eng = nc.gpsimd if pool_sp else nc.vector
                rl0 = sp * RSP
                pr2 = scratch.tile(
                    [128, RSP, C, O], f16, tag="prp" if pool_sp else "prd"
                )
                eng.tensor_mul(
                    pr2[:].rearrange("p r c (e two) -> p (r c) e two", two=2),
                    u_hat[:, rl0 : rl0 + RSP, :, :].rearrange(
                        "p r c (e two) -> p (r c) e two", two=2
                    ),
                    coef2[:, rl0 : rl0 + RSP, :, :]
                    .rearrange("p r c two -> p (r c) two")
                    .unsqueeze(2)
                    .broadcast_to((128, RSP * C, O // 2, 2)),
                )
                w = RSP
                prv = pr2[:].rearrange("p r c o -> p r (c o)")
                while w > 2:
                    h = w // 2
                    eng.tensor_add(prv[:, 0:h, :], prv[:, 0:h, :], prv[:, h:w, :])
                    w = h
                s_pair = s_pairP if pool_sp else s_pairD
                if (firstP if pool_sp else firstD):
                    eng.tensor_copy(s_pair[:], prv[:, 0:2, :])
                    if pool_sp:
                        firstP = False
                    else:
                        firstD = False
                else:
                    eng.tensor_add(s_pair[:], s_pair[:], prv[:, 0:2, :])
            # ---- combine 4 rq partition groups + o-pairs -> s_bt [32,C,O]
            nc.vector.tensor_add(
                s_pairD[:], s_pairD[:], s_pairP[:]
            )
            nc.vector.tensor_add(cmb[:], s_pairD[0:32, :, :], s_pairD[32:64, :, :])
            nc.vector.tensor_add(cmb[:], cmb[:], s_pairD[64:96, :, :])
            nc.vector.tensor_add(cmb[:], cmb[:], s_pairD[96:128, :, :])
            s_bt = sm.tile([32, C, O], f32, tag="sbt")
            nc.vector.tensor_add(
                s_bt[:].rearrange("p c o -> p (c o)"), cmb[:, 0, :], cmb[:, 1, :]
            )

            v_t = sm.tile([32, C, O], f32, tag="vt")
            squash(32, s_bt[:], v_t[:], sm)
            if t == 1:
                nc.vector.tensor_add(Vcb[bt][:], Vcb[bt][:], v_t[:])
            else:
                nc.sync.dma_start(
                    out=v_d[bt * BT : (bt + 1) * BT, :, :], in_=v_t[:]
                )

        if phase == "prep":
            pass
        elif phase == "einsum":
            with ExitStack() as es:
                mm_psum = es.enter_context(
                    tc.tile_pool(name="mmps0", bufs=2, space="PSUM")
                )
                einsum_tile(0, mm_psum)
        elif phase == "iter0":
            with ExitStack() as es:
                mm_psum = es.enter_context(
                    tc.tile_pool(name="mmps0", bufs=2, space="PSUM")
                )
                einsum_tile(0, mm_psum)
                nc.sync.dma_start(out=v_d[0:BT, :, :], in_=v0[0:32, :, :])
        else:
            for bt in range(2):
                with ExitStack() as es:
                    mm_psum = es.enter_context(
                        tc.tile_pool(name=f"mmps{bt}", bufs=2, space="PSUM")
                    )
                    einsum_tile(bt, mm_psum)
                for t in (1, 2):
                    routing_iter(bt, t)

    nc.compile()
    return nc


@functools.cache
def _get_nc():
    return build_bass()


@functools.cache
def _pack_cache():
    return {}


def _pack_inputs(u: np.ndarray, W: np.ndarray):
    import ml_dtypes

    bf = ml_dtypes.bfloat16
    # u: [B, R, I] per core slice -> ut [128=(8m+i), NK, B]
    # W: [R, C, I, O] -> wsb [128=(8m+i), NK, C, O]
    W5 = W.reshape(NK, 16, C, I, O)
    wsb = np.ascontiguousarray(
        W5.transpose(1, 3, 0, 2, 4).reshape(128, NK, C, O)
    ).astype(bf)
    m = (np.arange(128) // I)
    masks = [(m % 4 == j).astype(np.float32)[:, None, None] for j in range(4)]

    def pack_u(u_core):
        u5 = u_core.reshape(B, NK, 16, I)
        ut = np.ascontiguousarray(
            u5.transpose(2, 3, 1, 0).reshape(128, NK, B)
        )
        utz = np.stack([ut * msk for msk in masks], axis=0)
        return ut.astype(bf), utz.astype(bf)

    return wsb, pack_u


def kernel(u: np.ndarray, W: np.ndarray) -> np.ndarray:
    from concourse import bass_utils

    nc = _get_nc()
    W4 = np.ascontiguousarray(W.reshape(R, C, I, O)).astype(np.float32)
    wsb, pack_u = _pack_inputs(u, W4)
    in_maps = []
    for i in range(NCORES):
        ut, utz = pack_u(
            np.ascontiguousarray(u[i * B : (i + 1) * B]).astype(np.float32)
        )
        in_maps.append({"ut": ut, "utz": utz, "wsb": wsb})
    res = bass_utils.run_bass_kernel_spmd(
        nc, in_maps, core_ids=list(range(NCORES))
    )
    return np.concatenate([r["v"] for r in res.results], axis=0)


# revision 13
# speedup vs baseline: 1.8602x; 1.1239x over previous
"""DigitCaps (CapsNet dynamic routing) Trainium2 Bass kernel.

Full computation per batch element b:
    u_hat[r,c,o] = sum_i u[r,i] * W[r,c,i,o]            (einsum)
    b_log = 0; for 3 iters: coef = softmax_c(b_log); s = sum_r coef*u_hat
                v = squash(s); b_log += sum_o u_hat*v
Output: v from last iteration.  Identity used: b_log(t) = u_hat . Vcum(t)
where Vcum = sum of previous v's, so logits are recomputed from Vcum
each iteration instead of accumulated.

Sharding: data-parallel over batch, 512 -> 8 cores x 64.

Key cost-model-driven choices vs the naive version:
  - All operand layouts (u^T spread, masked uTz variants, W spread) are
    packed on the HOST and DMAed as single contiguous bf16 blocks: no
    strided gather DMAs, no on-device transposes or masking.
  - The z/s reductions run as halving ADD-trees in fp16 (TensorTensor,
    DVE 2x mode) instead of TensorReduce (which has no fast modes).
  - s-product keeps 2x mode via coef2 (coefficients duplicated in o-pairs
    so the broadcast AP stays packed in the last dim).
  - V broadcast and the rq-group combine use partition-offset DVE adds,
    not PE/PSUM, so PSUM belongs entirely to the einsum and the next
    batch-tile's einsum overlaps the tail of the current routing.
  - Elementwise work is split DVE (2x) / GPSIMD by span to balance
    engine occupancy; einsum PSUM evictions rotate DVE/ACT/GPSIMD.
"""

import sys

sys.path.insert(0, "/opt/trn_rl_repo")

import functools
from contextlib import ExitStack

import numpy as np

NCORES = 8
B = 64          # batch per core
BT = 32         # batch tile
R = 1152
C = 10
I = 8
O = 16
CO = C * O      # 160
NK = 72         # r-chunks of 16
RQ_K = 18       # k's per r-quartile
RL = 288        # r_loc per quartile (per partition)
RSP = 32        # rl span for routing passes
NSP = RL // RSP  # 9
POOL_SPANS = (0, 1)   # spans assigned to gpsimd (products + trees);
                      # first spans: the einsum produces their u_hat rows
                      # earliest, so the slow engine starts at pass begin
NORM_SPLIT = 192      # rl rows on DVE for the coef2 normalize


def _wslice(w):
    return slice(w * 32, (w + 1) * 32)


def build_bass(phase: str = "full"):
    import concourse.bass as bass
    import concourse.tile as tile
    from concourse import bacc, mybir

    f32 = mybir.dt.float32
    bf16 = mybir.dt.bfloat16
    f16 = mybir.dt.float16
    AX = mybir.AxisListType
    OP = mybir.AluOpType
    AF = mybir.ActivationFunctionType

    nc = bacc.Bacc(
        "TRN2",
        target_bir_lowering=False,
        debug=False,
        enable_asserts=False,
        num_devices=NCORES,
    )
    # Host-packed operands (bf16, contiguous):
    #   ut   [128, NK, B]    u^T spread: partition p = 8*m + i, r = 16k + m
    #   utz  [4, 128, NK, B] ut with only rows m%4 == j kept
    #   wsb  [128, NK, C, O] W spread: same partition map
    ut_d = nc.dram_tensor("ut", [128, NK, B], bf16, kind="ExternalInput").ap()
    utz_d = nc.dram_tensor("utz", [4, 128, NK, B], bf16, kind="ExternalInput").ap()
    w_d = nc.dram_tensor("wsb", [128, NK, C, O], bf16, kind="ExternalInput").ap()
    v_d = nc.dram_tensor("v", [B, C, O], f32, kind="ExternalOutput").ap()

    with tile.TileContext(nc) as tc, ExitStack() as ctx:
        # ---------------- persistent pools ----------------
        persist = ctx.enter_context(tc.tile_pool(name="persist", bufs=1))
        uTz0 = persist.tile([128, NK, B], bf16)
        uTz1 = persist.tile([128, NK, B], bf16)
        uTz2 = persist.tile([128, NK, B], bf16)
        uTz3 = persist.tile([128, NK, B], bf16)
        uTz = [uTz0, uTz1, uTz2, uTz3]
        W_sb = persist.tile([128, NK, C, O], bf16)

        rt = ctx.enter_context(tc.tile_pool(name="rt", bufs=1))
        logitsH = rt.tile([128, RL, C], f16)
        E = rt.tile([128, RL, C], bf16)
        den = rt.tile([128, RL], f32)
        coef2 = rt.tile([128, RL, C, 2], bf16)
        V_rep = rt.tile([128, C, O], bf16)
        s_pairD = rt.tile([128, 2, CO], f16)
        s_pairP = rt.tile([128, 2, CO], f16)
        cmb = rt.tile([32, 2, CO], f16)
        v0 = rt.tile([64, C, O], f32)
        Vcb0 = rt.tile([32, C, O], f32)
        Vcb1 = rt.tile([32, C, O], f32)
        Vcb = [Vcb0, Vcb1]
        sm = ctx.enter_context(tc.tile_pool(name="sm", bufs=1))

        def ecopy(which, out_ap, in_ap):
            # PSUM evictions go to DVE and (mostly) ACT; gpsimd is reserved
            # for its routing spans which must start as early as possible.
            if which % 3 == 0:
                nc.vector.tensor_copy(out_ap, in_ap)
            else:
                nc.scalar.copy(out_ap, in_ap)

        def squash(p, s_ap, out_ap, pool):
            # out = |s| / (1 + |s|^2) * s   per (partition, c)
            sq = pool.tile([p, C, O], f32, tag="sqt")
            nc.vector.tensor_mul(sq[:], s_ap, s_ap)
            ssum = pool.tile([p, C], f32, tag="sst")
            nc.vector.tensor_reduce(ssum[:], sq[:], axis=AX.X, op=OP.add)
            # sqrt(x) = exp(0.5*ln(x)): Ln and Exp share one activation
            # table with the softmax Exp, avoiding ACT table reloads.
            norm = pool.tile([p, C], f32, tag="snt")
            nc.scalar.activation(norm[:], ssum[:], AF.Ln)
            nc.scalar.activation(norm[:], norm[:], AF.Exp, scale=0.5)
            onep = pool.tile([p, C], f32, tag="sot")
            nc.scalar.add(onep[:], ssum[:], 1.0)
            rec = pool.tile([p, C], f32, tag="srt")
            nc.vector.reciprocal(rec[:], onep[:])
            fac = pool.tile([p, C], f32, tag="sft")
            nc.vector.tensor_mul(fac[:], norm[:], rec[:])
            nc.vector.tensor_mul(
                out_ap,
                s_ap,
                fac[:].unsqueeze(2).broadcast_to((p, C, O)),
            )

        def iter0(s0ps):
            s_all = rt.tile([64, C, O], f32)
            nc.scalar.mul(
                s_all[:], s0ps[:].rearrange("p (c o) -> p c o", c=C), 0.1
            )
            squash(64, s_all[:], v0[:], sm)
            nc.vector.tensor_copy(Vcb[0][:], v0[0:32, :, :])
            nc.scalar.copy(Vcb[1][:], v0[32:64, :, :])

        # ---------------- prep phase ----------------
        with ExitStack() as prep:
            pp = prep.enter_context(tc.tile_pool(name="prep", bufs=1))
            s0_pool = prep.enter_context(
                tc.tile_pool(name="s0psp", bufs=1, space="PSUM")
            )
            s0ps = s0_pool.tile([64, CO], f32)

            uT_full = pp.tile([128, NK, B], bf16)
            nc.sync.dma_start(out=uT_full[:], in_=ut_d)
            nc.scalar.dma_start(out=W_sb[:], in_=w_d)
            for j in range(4):
                deng = nc.sync if j % 2 == 0 else nc.scalar
                deng.dma_start(out=uTz[j][:], in_=utz_d[j])

            # iter-0 s matmul chain: s0 = sum_k uT_full[:,k,:].T @ W_sb[:,k]
            for k in range(NK):
                nc.tensor.matmul(
                    s0ps[:],
                    uT_full[:, k, :],
                    W_sb[:, k, :, :],
                    start=(k == 0),
                    stop=(k == NK - 1),
                )
            if phase != "prep":
                iter0(s0ps)

        # ---------------- main pools ----------------
        big = ctx.enter_context(tc.tile_pool(name="big", bufs=1))
        scratch = ctx.enter_context(tc.tile_pool(name="scratch", bufs=1))
        u_hat = big.tile([128, RL, C, O], bf16)      # 90 KB/part

        # ---------------- einsum: u_hat per batch tile ----------------
        def einsum_tile(bt, mm_psum):
            # Each MM isolates one r via the zero-masked uTz rows; the 4
            # row-groups (w) and 4 col-groups (rq) tile the PE array.
            for kl in range(RQ_K):
                for j in range(4):
                    pe_ps = mm_psum.tile([128, 4, 512], f32, tag="pe")
                    for rq in range(4):
                        k = rq * RQ_K + kl
                        for w in range(4):
                            nc.tensor.matmul(
                                pe_ps[rq * 32 : (rq + 1) * 32, w, 0:CO],
                                uTz[j][_wslice(w), k, bt * BT : (bt + 1) * BT],
                                W_sb[_wslice(w), k, :, :],
                                start=True,
                                stop=True,
                                tile_position=(w * 32, rq * 32),
                            )
                    rs0 = 16 * kl + j * 4
                    dst = u_hat[:, rs0 : rs0 + 4, :, :].rearrange(
                        "p r c o -> p r (c o)"
                    )
                    ecopy((kl * 4 + j) % 3, dst, pe_ps[:, :, 0:CO])

        # ---------------- routing iteration ----------------
        def vrep_update(bt):
            # V_rep[rq*32+b] = Vcb[bt][b] for all 4 rq groups (cast to bf16)
            nc.vector.tensor_copy(V_rep[0:32, :, :], Vcb[bt][:])
            nc.scalar.copy(V_rep[32:64, :, :], Vcb[bt][:])
            nc.vector.tensor_copy(V_rep[64:96, :, :], Vcb[bt][:])
            nc.scalar.copy(V_rep[96:128, :, :], Vcb[bt][:])

        # Pool spans are emitted first: gpsimd is ~4x slower per element,
        # so its spans define the pass tail and must start at pass begin.
        SPAN_ORDER = list(POOL_SPANS) + [
            sp for sp in range(NSP) if sp not in POOL_SPANS
        ]

        def routing_iter(bt, t):
            vrep_update(bt)
            # ---- z-pass: logits[p, rl, c] = sum_o u_hat * V_rep ----
            for sp in SPAN_ORDER:
                pool_sp = sp in POOL_SPANS
                eng = nc.gpsimd if pool_sp else nc.vector
                rl0 = sp * RSP
                pr = scratch.tile(
                    [128, RSP, C, O], f16, tag="prp" if pool_sp else "prd"
                )
                eng.tensor_mul(
                    pr[:],
                    u_hat[:, rl0 : rl0 + RSP, :, :],
                    V_rep[:].unsqueeze(1).broadcast_to((128, RSP, C, O)),
                )
                w = O
                while w > 2:
                    h = w // 2
                    eng.tensor_add(
                        pr[:, :, :, 0:h], pr[:, :, :, 0:h], pr[:, :, :, h:w]
                    )
                    w = h
                eng.tensor_add(
                    logitsH[:, rl0 : rl0 + RSP, :],
                    pr[:, :, :, 0],
                    pr[:, :, :, 1],
                )
            # ---- softmax over c (no max-shift: |z| << fp range) ----
            nc.scalar.activation(
                E[:].rearrange("p r c -> p (r c)"),
                logitsH[:].rearrange("p r c -> p (r c)"),
                AF.Exp,
            )
            nc.vector.tensor_reduce(den[:], E[:], axis=AX.X, op=OP.add)
            nc.vector.reciprocal(den[:], den[:])
            # coef2[p, rl, c, 0:2] = E * den^-1 duplicated in o-pairs
            ns = NORM_SPLIT
            nc.vector.tensor_mul(
                coef2[:, 0:ns, :, :],
                E[:, 0:ns, :].unsqueeze(3).broadcast_to((128, ns, C, 2)),
                den[:, 0:ns].unsqueeze(2).unsqueeze(3).broadcast_to(
                    (128, ns, C, 2)
                ),
            )
            nc.gpsimd.tensor_mul(
                coef2[:, ns:RL, :, :],
                E[:, ns:RL, :].unsqueeze(3).broadcast_to((128, RL - ns, C, 2)),
                den[:, ns:RL].unsqueeze(2).unsqueeze(3).broadcast_to(
                    (128, RL - ns, C, 2)
                ),
            )
            # ---- s-pass: s_pair*[p, 0:2, co] = sum_rl coef * u_hat ----
            firstD = True
            firstP = True
            for sp in SPAN_ORDER:
                pool_sp = sp in POOL_SPANS
                eng = nc.gpsimd if pool_sp else nc.vector
                rl0 = sp * RSP
                pr2 = scratch.tile(
                    [128, RSP, C, O], f16, tag="prp" if pool_sp else "prd"
                )
                eng.tensor_mul(
                    pr2[:].rearrange("p r c (e two) -> p (r c) e two", two=2),
                    u_hat[:, rl0 : rl0 + RSP, :, :].rearrange(
                        "p r c (e two) -> p (r c) e two", two=2
                    ),
                    coef2[:, rl0 : rl0 + RSP, :, :]
                    .rearrange("p r c two -> p (r c) two")
                    .unsqueeze(2)
                    .broadcast_to((128, RSP * C, O // 2, 2)),
                )
                w = RSP
                prv = pr2[:].rearrange("p r c o -> p r (c o)")
                while w > 2:
                    h = w // 2
                    eng.tensor_add(prv[:, 0:h, :], prv[:, 0:h, :], prv[:, h:w, :])
                    w = h
                s_pair = s_pairP if pool_sp else s_pairD
                if (firstP if pool_sp else firstD):
                    eng.tensor_copy(s_pair[:], prv[:, 0:2, :])
                    if pool_sp:
                        firstP = False
                    else:
                        firstD = False
                else:
                    eng.tensor_add(s_pair[:], s_pair[:], prv[:, 0:2, :])
            # ---- combine 4 rq partition groups + o-pairs -> s_bt [32,C,O]
            nc.vector.tensor_add(
                s_pairD[:], s_pairD[:], s_pairP[:]
            )
            nc.vector.tensor_add(cmb[:], s_pairD[0:32, :, :], s_pairD[32:64, :, :])
            nc.vector.tensor_add(cmb[:], cmb[:], s_pairD[64:96, :, :])
            nc.vector.tensor_add(cmb[:], cmb[:], s_pairD[96:128, :, :])
            s_bt = sm.tile([32, C, O], f32, tag="sbt")
            nc.vector.tensor_add(
                s_bt[:].rearrange("p c o -> p (c o)"), cmb[:, 0, :], cmb[:, 1, :]
            )

            v_t = sm.tile([32, C, O], f32, tag="vt")
            squash(32, s_bt[:], v_t[:], sm)
            if t == 1:
                nc.vector.tensor_add(Vcb[bt][:], Vcb[bt][:], v_t[:])
            else:
                nc.sync.dma_start(
                    out=v_d[bt * BT : (bt + 1) * BT, :, :], in_=v_t[:]
                )

        if phase == "prep":
            pass
        elif phase == "einsum":
            with ExitStack() as es:
                mm_psum = es.enter_context(
                    tc.tile_pool(name="mmps0", bufs=2, space="PSUM")
                )
                einsum_tile(0, mm_psum)
        elif phase == "iter0":
            with ExitStack() as es:
                mm_psum = es.enter_context(
                    tc.tile_pool(name="mmps0", bufs=2, space="PSUM")
                )
                einsum_tile(0, mm_psum)
                nc.sync.dma_start(out=v_d[0:BT, :, :], in_=v0[0:32, :, :])
        else:
            for bt in range(2):
                with ExitStack() as es:
                    mm_psum = es.enter_context(
                        tc.tile_pool(name=f"mmps{bt}", bufs=2, space="PSUM")
                    )
                    einsum_tile(bt, mm_psum)
                for t in (1, 2):
                    routing_iter(bt, t)

    nc.compile()
    return nc


@functools.cache
def _get_nc():
    return build_bass()


@functools.cache
def _pack_cache():
    return {}


def _pack_inputs(u: np.ndarray, W: np.ndarray):
    import ml_dtypes

    bf = ml_dtypes.bfloat16
    # u: [B, R, I] per core slice -> ut [128=(8m+i), NK, B]
    # W: [R, C, I, O] -> wsb [128=(8m+i), NK, C, O]
    W5 = W.reshape(NK, 16, C, I, O)
    wsb = np.ascontiguousarray(
        W5.transpose(1, 3, 0, 2, 4).reshape(128, NK, C, O)
    ).astype(bf)
    m = (np.arange(128) // I)
    masks = [(m % 4 == j).astype(np.float32)[:, None, None] for j in range(4)]

    def pack_u(u_core):
        u5 = u_core.reshape(B, NK, 16, I)
        ut = np.ascontiguousarray(
            u5.transpose(2, 3, 1, 0).reshape(128, NK, B)
        )
        utz = np.stack([ut * msk for msk in masks], axis=0)
        return ut.astype(bf), utz.astype(bf)

    return wsb, pack_u


def kernel(u: np.ndarray, W: np.ndarray) -> np.ndarray:
    from concourse import bass_utils

    nc = _get_nc()
    W4 = np.ascontiguousarray(W.reshape(R, C, I, O)).astype(np.float32)
    wsb, pack_u = _pack_inputs(u, W4)
    in_maps = []
    for i in range(NCORES):
        ut, utz = pack_u(
            np.ascontiguousarray(u[i * B : (i + 1) * B]).astype(np.float32)
        )
        in_maps.append({"ut": ut, "utz": utz, "wsb": wsb})
    res = bass_utils.run_bass_kernel_spmd(
        nc, in_maps, core_ids=list(range(NCORES))
    )
    return np.concatenate([r["v"] for r in res.results], axis=0)
